# revision 7
# baseline (speedup 1.0000x reference)
"""FlowNet-style correlation layer (MAX_DISPLACEMENT=4, 81 channels) on 8 TRN2 cores.

Strategy
--------
Data-parallel over batch N=8 -> 1 sample per NeuronCore.

Per core, for each 16x8 spatial patch of data1 (the "stationary" block, M=128
positions) we matmul against the surrounding 24x16 patch of zero-padded data2
(the "moving" block, N=384 columns), contracting over C=256 in two K=128
chunks accumulated in PSUM.  The [128, 384] PSUM tile then contains, for every
stationary position m=(g,j), all 81 correlation values at free offsets
(g+dyp)*16 + (j+dxp) -- a banded/diagonal layout that no on-chip engine can
re-align cheaply (access patterns cannot vary offsets per partition).  So the
device streams the scaled band tiles out as fp16 and the final diagonal
gather happens on the host in numpy (a single fancy-index per sample).

fp16 is used for inputs and band outputs: full-rate on the PE (fp32 PSUM
accumulation), and it halves DMA volume.  Error vs the fp32 reference is
~1e-3 relative.
"""

import numpy as np

C, H, W = 256, 96, 160
PAD = 4
NG = 9  # displacement grid width (2*4+1)
Q = NG * NG  # 81 output channels
GB, BB = 8, 16  # stationary block: GB rows x BB cols = 128 positions
TT, UU = GB + 2 * PAD, BB + 2 * PAD  # moving block: 16 rows x 24 cols
NBY, NBX = H // GB, W // BB  # 12 x 10 = 120 blocks
NBLK = NBY * NBX
NMOV = TT * UU  # 384 moving columns per matmul
HP, WP = H + 2 * PAD, W + 2 * PAD
N_CORES = 8

_CACHE = {}


def _build_bass():
    import concourse.bass as bass  # noqa: F401
    import concourse.mybir as mybir
    import concourse.tile as tile
    from concourse import bacc

    fp16 = mybir.dt.float16
    fp32 = mybir.dt.float32

    nc = bacc.Bacc("TRN2", target_bir_lowering=False, debug=False)

    # d1b: data1 tiled into exact 8x16 blocks -> [C, NBLK, 128]
    # d2b: padded data2 in 10 overlapping x-slabs -> [C, NBX, HP, UU]
    d1b = nc.dram_tensor("d1b", [C, NBLK, 128], fp16, kind="ExternalInput").ap()
    d2b = nc.dram_tensor("d2b", [C, NBX, HP, UU], fp16, kind="ExternalInput").ap()
    bands = nc.dram_tensor(
        "bands", [NBLK, 128, NMOV], fp16, kind="ExternalOutput"
    ).ap()

    with tile.TileContext(nc) as tc:
        with (
            tc.tile_pool(name="inp", bufs=1) as inp_pool,
            tc.tile_pool(name="ps", bufs=8, space="PSUM") as ps_pool,
            tc.tile_pool(name="ob", bufs=8) as ob_pool,
        ):
            d1_sb = inp_pool.tile([128, 2, NBLK, 128], fp16, tag="d1sb")
            d2_sb = inp_pool.tile([128, 2, NBX, HP, UU], fp16, tag="d2sb")
            nc.sync.dma_start(
                out=d1_sb[:], in_=d1b.rearrange("(k p) b m -> p k b m", p=128)
            )
            nc.sync.dma_start(
                out=d2_sb[:], in_=d2b.rearrange("(k p) b y u -> p k b y u", p=128)
            )

            for by in range(NBY):
                for bx in range(NBX):
                    blk = by * NBX + bx
                    y0 = by * GB
                    ps = ps_pool.tile([128, NMOV], fp32)
                    for k in range(2):
                        lhsT = d1_sb[:, k, blk, :]
                        rhs = d2_sb[:, k, bx, y0 : y0 + TT, :]
                        nc.tensor.matmul(
                            ps[:], lhsT, rhs, start=(k == 0), stop=(k == 1)
                        )
                    ob = ob_pool.tile([128, NMOV], fp16)
                    nc.vector.tensor_scalar_mul(ob[:], ps[:], 1.0 / C)
                    nc.sync.dma_start(out=bands[blk], in_=ob[:])

    nc.compile()
    return nc


def _get_nc():
    if "nc" not in _CACHE:
        _CACHE["nc"] = _build_bass()
    return _CACHE["nc"]


def _gather_index():
    """Flat indices into a [128, NMOV] band tile: idx[g, j, dyp, dxp]."""
    if "idx" not in _CACHE:
        g = np.arange(GB)[:, None, None, None]
        j = np.arange(BB)[None, :, None, None]
        dyp = np.arange(NG)[None, None, :, None]
        dxp = np.arange(NG)[None, None, None, :]
        m = g * BB + j
        n = (g + dyp) * UU + (j + dxp)
        _CACHE["idx"] = (m * NMOV + n).reshape(-1)
    return _CACHE["idx"]


def _extract(bands_u8):
    """[NBLK, 128, NMOV] fp16 band tiles -> [Q, H, W] fp32 output."""
    flat = np.ascontiguousarray(bands_u8).reshape(NBLK, 128 * NMOV)
    sub = flat[:, _gather_index()].astype(np.float32)
    sub = sub.reshape(NBY, NBX, GB, BB, NG, NG)
    return sub.transpose(4, 5, 0, 2, 1, 3).reshape(Q, H, W)


def prepare_inputs(data1, data2):
    """Full [N,C,H,W] fp32 inputs -> per-core in_maps (pre-blocked fp16)."""
    d1h = np.asarray(data1, dtype=np.float16)
    d2h = np.pad(
        np.asarray(data2, dtype=np.float16),
        ((0, 0), (0, 0), (PAD, PAD), (PAD, PAD)),
    )
    # [N, C, H, W] -> [N, C, NBY, GB, NBX, BB] -> [N, C, NBLK, 128]
    d1t = (
        d1h.reshape(N_CORES, C, NBY, GB, NBX, BB)
        .transpose(0, 1, 2, 4, 3, 5)
        .reshape(N_CORES, C, NBLK, 128)
    )
    # [N, C, HP, WP] -> x-slabs [N, C, NBX, HP, UU]
    sl = np.lib.stride_tricks.sliding_window_view(d2h, UU, axis=3)[
        :, :, :, :: BB, :
    ]  # [N, C, HP, NBX, UU]
    d2t = sl.transpose(0, 1, 3, 2, 4)
    return [
        {
            "d1b": np.ascontiguousarray(d1t[i]),
            "d2b": np.ascontiguousarray(d2t[i]),
        }
        for i in range(N_CORES)
    ]


def _get_runner():
    """Cached jit'd shard_map executable: f(*concat_inputs) -> concat outputs.

    Modeled on concourse.bass2jax.run_bass_via_pjrt, but built once and
    reusable so repeated kernel() calls (and timing loops) skip re-tracing.
    """
    if "runner" in _CACHE:
        return _CACHE["runner"]

    import jax
    from jax.sharding import Mesh, PartitionSpec
    from jax.experimental.shard_map import shard_map
    import concourse.mybir as mybir
    from concourse import bass2jax

    bass2jax.install_neuronx_cc_hook()
    nc = _get_nc()

    partition_name = nc.partition_id_tensor.name if nc.partition_id_tensor else None
    in_names, out_names, out_avals = [], [], []
    for alloc in nc.m.functions[0].allocations:
        if not isinstance(alloc, mybir.MemoryLocationSet):
            continue
        name = alloc.memorylocations[0].name
        if alloc.kind == "ExternalInput":
            if name != partition_name:
                in_names.append(name)
        elif alloc.kind == "ExternalOutput":
            out_names.append(name)
            out_avals.append(
                jax.core.ShapedArray(
                    tuple(alloc.tensor_shape), mybir.dt.np(alloc.dtype)
                )
            )
    n_params = len(in_names)
    all_in_names = in_names + out_names
    if partition_name is not None:
        all_in_names = all_in_names + [partition_name]

    def _body(*args):
        operands = list(args)
        if partition_name is not None:
            operands.append(bass2jax.partition_id_tensor())
        outs = bass2jax._bass_exec_p.bind(
            *operands,
            out_avals=tuple(out_avals),
            in_names=tuple(all_in_names),
            out_names=tuple(out_names),
            lowering_input_output_aliases=(),
            sim_require_finite=True,
            sim_require_nnan=True,
            nc=nc,
        )
        return tuple(outs)

    devices = jax.devices()[:N_CORES]
    mesh = Mesh(np.asarray(devices), ("core",))
    n_outs = len(out_names)
    sharded = jax.jit(
        shard_map(
            _body,
            mesh=mesh,
            in_specs=(PartitionSpec("core"),) * (n_params + n_outs),
            out_specs=(PartitionSpec("core"),) * n_outs,
            check_rep=False,
        ),
        keep_unused=True,
    )
    runner = {
        "fn": sharded,
        "in_names": in_names,
        "out_names": out_names,
        "out_avals": out_avals,
        "mesh": mesh,
    }
    _CACHE["runner"] = runner
    return runner


def run_hw(in_maps):
    """Execute on 8 cores; returns list of per-core {name: np.ndarray}."""
    r = _get_runner()
    concat_in = [
        np.concatenate([m[name] for m in in_maps], axis=0) for name in r["in_names"]
    ]
    concat_zeros = [
        np.zeros((N_CORES * a.shape[0], *a.shape[1:]), a.dtype)
        for a in r["out_avals"]
    ]
    out_arrs = r["fn"](*concat_in, *concat_zeros)
    return [
        {
            name: np.asarray(out_arrs[i]).reshape(
                N_CORES, *r["out_avals"][i].shape
            )[c]
            for i, name in enumerate(r["out_names"])
        }
        for c in range(N_CORES)
    ]


def kernel(data1, data2):
    in_maps = prepare_inputs(data1, data2)
    results = run_hw(in_maps)
    out = np.stack([_extract(r["bands"]) for r in results])
    return out.astype(np.float32)


# revision 12
# speedup vs baseline: 420.8538x; 420.8538x over previous
"""FlowNet-style correlation layer (MAX_DISPLACEMENT=4, 81 channels) on 8 TRN2 cores.

Strategy
--------
Data-parallel over batch N=8 -> 1 sample per NeuronCore.

Per core, for each 16x8 spatial patch of data1 (the "stationary" block, M=128
positions) we matmul against the surrounding 24x16 patch of zero-padded data2
(the "moving" block, N=384 columns), contracting over C=256 in two K=128
chunks accumulated in PSUM.  The [128, 384] PSUM tile then contains, for every
stationary position m=(g,j), all 81 correlation values at free offsets
(g+dyp)*16 + (j+dxp) -- a banded/diagonal layout that no on-chip engine can
re-align cheaply (access patterns cannot vary offsets per partition).  So the
device streams the scaled band tiles out as fp16 and the final diagonal
gather happens on the host in numpy (a single fancy-index per sample).

fp16 is used for inputs and band outputs: full-rate on the PE (fp32 PSUM
accumulation), and it halves DMA volume.  Error vs the fp32 reference is
~1e-3 relative.
"""

import numpy as np

C, H, W = 256, 96, 160
PAD = 4
NG = 9  # displacement grid width (2*4+1)
Q = NG * NG  # 81 output channels
GB, BB = 8, 16  # stationary block: GB rows x BB cols = 128 positions
TT, UU = GB + 2 * PAD, BB + 2 * PAD  # moving block: 16 rows x 24 cols
NBY, NBX = H // GB, W // BB  # 12 x 10 = 120 blocks
NBLK = NBY * NBX
NMOV = TT * UU  # 384 moving columns per matmul
HP, WP = H + 2 * PAD, W + 2 * PAD
N_CORES = 8

_CACHE = {}


def _build_bass(reps=1):
    import contextlib

    import concourse.bass as bass  # noqa: F401
    import concourse.mybir as mybir
    import concourse.tile as tile
    from concourse import bacc

    fp16 = mybir.dt.float16
    fp32 = mybir.dt.float32

    nc = bacc.Bacc("TRN2", target_bir_lowering=False, debug=False)

    # d1b: data1 tiled into exact 8x16 blocks -> [C, NBLK, 128]
    # d2b: padded data2 in 10 overlapping x-slabs -> [C, NBX, HP, UU]
    d1b = nc.dram_tensor("d1b", [C, NBLK, 128], fp16, kind="ExternalInput").ap()
    d2b = nc.dram_tensor("d2b", [C, NBX, HP, UU], fp16, kind="ExternalInput").ap()
    bands = nc.dram_tensor(
        "bands", [NBLK, 128, NMOV], fp16, kind="ExternalOutput"
    ).ap()

    with tile.TileContext(nc) as tc:
        with (
            tc.tile_pool(name="inp", bufs=1) as inp_pool,
            tc.tile_pool(name="ps", bufs=8, space="PSUM") as ps_pool,
            tc.tile_pool(name="ob", bufs=8) as ob_pool,
        ):
            loop = tc.For_i(0, reps, 1) if reps > 1 else contextlib.nullcontext()
            with loop:
                d1_sb = inp_pool.tile([128, 2, NBLK, 128], fp16, tag="d1sb")
                d2_sb = inp_pool.tile([128, 2, NBX, HP, UU], fp16, tag="d2sb")
                nc.sync.dma_start(
                    out=d1_sb[:], in_=d1b.rearrange("(k p) b m -> p k b m", p=128)
                )
                nc.sync.dma_start(
                    out=d2_sb[:], in_=d2b.rearrange("(k p) b y u -> p k b y u", p=128)
                )

                for by in range(NBY):
                    for bx in range(NBX):
                        blk = by * NBX + bx
                        y0 = by * GB
                        ps = ps_pool.tile([128, NMOV], fp32)
                        for k in range(2):
                            lhsT = d1_sb[:, k, blk, :]
                            rhs = d2_sb[:, k, bx, y0 : y0 + TT, :]
                            nc.tensor.matmul(
                                ps[:], lhsT, rhs, start=(k == 0), stop=(k == 1)
                            )
                        ob = ob_pool.tile([128, NMOV], fp16)
                        nc.vector.tensor_scalar_mul(ob[:], ps[:], 1.0 / C)
                        nc.sync.dma_start(out=bands[blk], in_=ob[:])

    nc.compile()
    return nc


def _get_nc(reps=1):
    key = ("nc", reps)
    if key not in _CACHE:
        _CACHE[key] = _build_bass(reps)
    return _CACHE[key]


def _gather_index():
    """Flat indices into a [128, NMOV] band tile: idx[g, j, dyp, dxp]."""
    if "idx" not in _CACHE:
        g = np.arange(GB)[:, None, None, None]
        j = np.arange(BB)[None, :, None, None]
        dyp = np.arange(NG)[None, None, :, None]
        dxp = np.arange(NG)[None, None, None, :]
        m = g * BB + j
        n = (g + dyp) * UU + (j + dxp)
        _CACHE["idx"] = (m * NMOV + n).reshape(-1)
    return _CACHE["idx"]


def _extract(bands_u8):
    """[NBLK, 128, NMOV] fp16 band tiles -> [Q, H, W] fp32 output."""
    flat = np.ascontiguousarray(bands_u8).reshape(NBLK, 128 * NMOV)
    sub = flat[:, _gather_index()].astype(np.float32)
    sub = sub.reshape(NBY, NBX, GB, BB, NG, NG)
    return sub.transpose(4, 5, 0, 2, 1, 3).reshape(Q, H, W)


def prepare_inputs(data1, data2):
    """Full [N,C,H,W] fp32 inputs -> per-core in_maps (pre-blocked fp16)."""
    d1h = np.asarray(data1, dtype=np.float16)
    d2h = np.pad(
        np.asarray(data2, dtype=np.float16),
        ((0, 0), (0, 0), (PAD, PAD), (PAD, PAD)),
    )
    # [N, C, H, W] -> [N, C, NBY, GB, NBX, BB] -> [N, C, NBLK, 128]
    d1t = (
        d1h.reshape(N_CORES, C, NBY, GB, NBX, BB)
        .transpose(0, 1, 2, 4, 3, 5)
        .reshape(N_CORES, C, NBLK, 128)
    )
    # [N, C, HP, WP] -> x-slabs [N, C, NBX, HP, UU]
    sl = np.lib.stride_tricks.sliding_window_view(d2h, UU, axis=3)[
        :, :, :, :: BB, :
    ]  # [N, C, HP, NBX, UU]
    d2t = sl.transpose(0, 1, 3, 2, 4)
    return [
        {
            "d1b": np.ascontiguousarray(d1t[i]),
            "d2b": np.ascontiguousarray(d2t[i]),
        }
        for i in range(N_CORES)
    ]


def _get_runner(reps=1):
    """Cached jit'd shard_map executable: f(*concat_inputs) -> concat outputs.

    Modeled on concourse.bass2jax.run_bass_via_pjrt, but built once and
    reusable so repeated kernel() calls (and timing loops) skip re-tracing.
    """
    rkey = ("runner", reps)
    if rkey in _CACHE:
        return _CACHE[rkey]

    import jax
    from jax.sharding import Mesh, PartitionSpec
    from jax.experimental.shard_map import shard_map
    import concourse.mybir as mybir
    from concourse import bass2jax

    bass2jax.install_neuronx_cc_hook()
    nc = _get_nc(reps)

    partition_name = nc.partition_id_tensor.name if nc.partition_id_tensor else None
    in_names, out_names, out_avals = [], [], []
    for alloc in nc.m.functions[0].allocations:
        if not isinstance(alloc, mybir.MemoryLocationSet):
            continue
        name = alloc.memorylocations[0].name
        if alloc.kind == "ExternalInput":
            if name != partition_name:
                in_names.append(name)
        elif alloc.kind == "ExternalOutput":
            out_names.append(name)
            out_avals.append(
                jax.core.ShapedArray(
                    tuple(alloc.tensor_shape), mybir.dt.np(alloc.dtype)
                )
            )
    n_params = len(in_names)
    all_in_names = in_names + out_names
    if partition_name is not None:
        all_in_names = all_in_names + [partition_name]

    def _body(*args):
        operands = list(args)
        if partition_name is not None:
            operands.append(bass2jax.partition_id_tensor())
        outs = bass2jax._bass_exec_p.bind(
            *operands,
            out_avals=tuple(out_avals),
            in_names=tuple(all_in_names),
            out_names=tuple(out_names),
            lowering_input_output_aliases=(),
            sim_require_finite=True,
            sim_require_nnan=True,
            nc=nc,
        )
        return tuple(outs)

    devices = jax.devices()[:N_CORES]
    mesh = Mesh(np.asarray(devices), ("core",))
    n_outs = len(out_names)
    sharded = jax.jit(
        shard_map(
            _body,
            mesh=mesh,
            in_specs=(PartitionSpec("core"),) * (n_params + n_outs),
            out_specs=(PartitionSpec("core"),) * n_outs,
            check_rep=False,
        ),
        keep_unused=True,
    )
    runner = {
        "fn": sharded,
        "in_names": in_names,
        "out_names": out_names,
        "out_avals": out_avals,
        "mesh": mesh,
    }
    _CACHE[rkey] = runner
    return runner


def run_hw(in_maps):
    """Execute on 8 cores; returns list of per-core {name: np.ndarray}."""
    r = _get_runner()
    concat_in = [
        np.concatenate([m[name] for m in in_maps], axis=0) for name in r["in_names"]
    ]
    concat_zeros = [
        np.zeros((N_CORES * a.shape[0], *a.shape[1:]), a.dtype)
        for a in r["out_avals"]
    ]
    out_arrs = r["fn"](*concat_in, *concat_zeros)
    return [
        {
            name: np.asarray(out_arrs[i]).reshape(
                N_CORES, *r["out_avals"][i].shape
            )[c]
            for i, name in enumerate(r["out_names"])
        }
        for c in range(N_CORES)
    ]


def kernel(data1, data2):
    in_maps = prepare_inputs(data1, data2)
    results = run_hw(in_maps)
    out = np.stack([_extract(r["bands"]) for r in results])
    return out.astype(np.float32)


# revision 20
# speedup vs baseline: 567.4492x; 1.3483x over previous
"""FlowNet-style correlation layer (MAX_DISPLACEMENT=4, 81 channels) on 8 TRN2 cores.

Strategy
--------
Data-parallel over batch N=8 -> 1 sample per NeuronCore.

Per core, for each 16x8 spatial patch of data1 (the "stationary" block, M=128
positions) we matmul against the surrounding 24x16 patch of zero-padded data2
(the "moving" block, N=384 columns), contracting over C=256 in two K=128
chunks accumulated in PSUM.  The [128, 384] PSUM tile then contains, for every
stationary position m=(g,j), all 81 correlation values at free offsets
(g+dyp)*16 + (j+dxp) -- a banded/diagonal layout that no on-chip engine can
re-align cheaply (access patterns cannot vary offsets per partition).  So the
device streams the scaled band tiles out as fp16 and the final diagonal
gather happens on the host in numpy (a single fancy-index per sample).

fp16 is used for inputs and band outputs: full-rate on the PE (fp32 PSUM
accumulation), and it halves DMA volume.  Error vs the fp32 reference is
~1e-3 relative.
"""

import numpy as np

C, H, W = 256, 96, 160
PAD = 4
NG = 9  # displacement grid width (2*4+1)
Q = NG * NG  # 81 output channels
GB, BB = 8, 16  # stationary block: GB rows x BB cols = 128 positions
TT, UU = GB + 2 * PAD, BB + 2 * PAD  # moving block: 16 rows x 24 cols
NBY, NBX = H // GB, W // BB  # 12 x 10 = 120 blocks
NBLK = NBY * NBX
NMOV = TT * UU  # 384 moving columns per matmul
HP, WP = H + 2 * PAD, W + 2 * PAD
N_CORES = 8

_CACHE = {}


def _build_bass(reps=1):
    import contextlib

    import concourse.bass as bass  # noqa: F401
    import concourse.mybir as mybir
    import concourse.tile as tile
    from concourse import bacc

    fp16 = mybir.dt.float16
    fp32 = mybir.dt.float32

    nc = bacc.Bacc("TRN2", target_bir_lowering=False, debug=False)

    # d1b: data1 tiled into exact 8x16 blocks, bx-major -> [C, NBX, NBY, 128]
    # d2b: padded data2 in NBX overlapping x-slabs -> [C, NBX, HP, UU]
    d1b = nc.dram_tensor(
        "d1b", [C, NBX, NBY, 128], fp16, kind="ExternalInput"
    ).ap()
    d2b = nc.dram_tensor("d2b", [C, NBX, HP, UU], fp16, kind="ExternalInput").ap()
    bands = nc.dram_tensor(
        "bands", [NBX, 128, NBY, NMOV], fp16, kind="ExternalOutput"
    ).ap()

    d1r = d1b.rearrange("(k p) bx by m -> p k bx by m", p=128)
    d2r = d2b.rearrange("(k p) bx y u -> p k bx y u", p=128)

    with tile.TileContext(nc) as tc:
        with (
            tc.tile_pool(name="in1", bufs=4) as in1_pool,
            tc.tile_pool(name="in2", bufs=4) as in2_pool,
            tc.tile_pool(name="ps", bufs=8, space="PSUM") as ps_pool,
            tc.tile_pool(name="ob", bufs=3) as ob_pool,
        ):
            loop = tc.For_i(0, reps, 1) if reps > 1 else contextlib.nullcontext()
            with loop:
                # Column-pipelined: load per-bx chunks; compute overlaps the
                # next columns' loads via pool multi-buffering.
                for bx in range(NBX):
                    d1_sb = in1_pool.tile([128, 2, NBY, 128], fp16, tag="d1sb")
                    d2_sb = in2_pool.tile([128, 2, HP, UU], fp16, tag="d2sb")
                    # Separate engine queues so the DMAs overlap each other.
                    nc.gpsimd.dma_start(out=d1_sb[:], in_=d1r[:, :, bx])
                    nc.sync.dma_start(out=d2_sb[:], in_=d2r[:, :, bx])
                    ob = ob_pool.tile([128, NBY, NMOV], fp16, tag="ob")
                    for by in range(NBY):
                        y0 = by * GB
                        ps = ps_pool.tile([128, NMOV], fp32)
                        for k in range(2):
                            lhsT = d1_sb[:, k, by, :]
                            rhs = d2_sb[:, k, y0 : y0 + TT, :]
                            nc.tensor.matmul(
                                ps[:], lhsT, rhs, start=(k == 0), stop=(k == 1)
                            )
                        if by % 3 == 2:
                            nc.scalar.mul(ob[:, by, :], ps[:], 1.0 / C)
                        else:
                            nc.vector.tensor_scalar_mul(ob[:, by, :], ps[:], 1.0 / C)
                    nc.scalar.dma_start(out=bands[bx], in_=ob[:])

    nc.compile()
    return nc


def _get_nc(reps=1):
    key = ("nc", reps)
    if key not in _CACHE:
        _CACHE[key] = _build_bass(reps)
    return _CACHE[key]


def _gather_index():
    """Flat indices into a [128, NMOV] band tile: idx[g, j, dyp, dxp]."""
    if "idx" not in _CACHE:
        g = np.arange(GB)[:, None, None, None]
        j = np.arange(BB)[None, :, None, None]
        dyp = np.arange(NG)[None, None, :, None]
        dxp = np.arange(NG)[None, None, None, :]
        m = g * BB + j
        n = (g + dyp) * UU + (j + dxp)
        _CACHE["idx"] = (m * NMOV + n).reshape(-1)
    return _CACHE["idx"]


def _extract(bands_u8):
    """[NBX, 128, NBY, NMOV] fp16 band tiles -> [Q, H, W] fp32 output."""
    arr = np.ascontiguousarray(bands_u8).transpose(0, 2, 1, 3)  # [NBX, NBY, 128, NMOV]
    flat = np.ascontiguousarray(arr).reshape(NBLK, 128 * NMOV)
    sub = flat[:, _gather_index()].astype(np.float32)
    sub = sub.reshape(NBX, NBY, GB, BB, NG, NG)
    return sub.transpose(4, 5, 1, 2, 0, 3).reshape(Q, H, W)


def prepare_inputs(data1, data2):
    """Full [N,C,H,W] fp32 inputs -> per-core in_maps (pre-blocked fp16)."""
    d1h = np.asarray(data1, dtype=np.float16)
    d2h = np.pad(
        np.asarray(data2, dtype=np.float16),
        ((0, 0), (0, 0), (PAD, PAD), (PAD, PAD)),
    )
    # [N, C, H, W] -> [N, C, NBY, GB, NBX, BB] -> bx-major [N, C, NBX, NBY, 128]
    d1t = (
        d1h.reshape(N_CORES, C, NBY, GB, NBX, BB)
        .transpose(0, 1, 4, 2, 3, 5)
        .reshape(N_CORES, C, NBX, NBY, 128)
    )
    # [N, C, HP, WP] -> x-slabs [N, C, NBX, HP, UU]
    sl = np.lib.stride_tricks.sliding_window_view(d2h, UU, axis=3)[
        :, :, :, :: BB, :
    ]  # [N, C, HP, NBX, UU]
    d2t = sl.transpose(0, 1, 3, 2, 4)
    return [
        {
            "d1b": np.ascontiguousarray(d1t[i]),
            "d2b": np.ascontiguousarray(d2t[i]),
        }
        for i in range(N_CORES)
    ]


def _get_runner(reps=1):
    """Cached jit'd shard_map executable: f(*concat_inputs) -> concat outputs.

    Modeled on concourse.bass2jax.run_bass_via_pjrt, but built once and
    reusable so repeated kernel() calls (and timing loops) skip re-tracing.
    """
    rkey = ("runner", reps)
    if rkey in _CACHE:
        return _CACHE[rkey]

    import jax
    from jax.sharding import Mesh, PartitionSpec
    from jax.experimental.shard_map import shard_map
    import concourse.mybir as mybir
    from concourse import bass2jax

    bass2jax.install_neuronx_cc_hook()
    nc = _get_nc(reps)

    partition_name = nc.partition_id_tensor.name if nc.partition_id_tensor else None
    in_names, out_names, out_avals = [], [], []
    for alloc in nc.m.functions[0].allocations:
        if not isinstance(alloc, mybir.MemoryLocationSet):
            continue
        name = alloc.memorylocations[0].name
        if alloc.kind == "ExternalInput":
            if name != partition_name:
                in_names.append(name)
        elif alloc.kind == "ExternalOutput":
            out_names.append(name)
            out_avals.append(
                jax.core.ShapedArray(
                    tuple(alloc.tensor_shape), mybir.dt.np(alloc.dtype)
                )
            )
    n_params = len(in_names)
    all_in_names = in_names + out_names
    if partition_name is not None:
        all_in_names = all_in_names + [partition_name]

    def _body(*args):
        operands = list(args)
        if partition_name is not None:
            operands.append(bass2jax.partition_id_tensor())
        outs = bass2jax._bass_exec_p.bind(
            *operands,
            out_avals=tuple(out_avals),
            in_names=tuple(all_in_names),
            out_names=tuple(out_names),
            lowering_input_output_aliases=(),
            sim_require_finite=True,
            sim_require_nnan=True,
            nc=nc,
        )
        return tuple(outs)

    devices = jax.devices()[:N_CORES]
    mesh = Mesh(np.asarray(devices), ("core",))
    n_outs = len(out_names)
    sharded = jax.jit(
        shard_map(
            _body,
            mesh=mesh,
            in_specs=(PartitionSpec("core"),) * (n_params + n_outs),
            out_specs=(PartitionSpec("core"),) * n_outs,
            check_rep=False,
        ),
        keep_unused=True,
    )
    runner = {
        "fn": sharded,
        "in_names": in_names,
        "out_names": out_names,
        "out_avals": out_avals,
        "mesh": mesh,
    }
    _CACHE[rkey] = runner
    return runner


def run_hw(in_maps):
    """Execute on 8 cores; returns list of per-core {name: np.ndarray}."""
    r = _get_runner()
    concat_in = [
        np.concatenate([m[name] for m in in_maps], axis=0) for name in r["in_names"]
    ]
    concat_zeros = [
        np.zeros((N_CORES * a.shape[0], *a.shape[1:]), a.dtype)
        for a in r["out_avals"]
    ]
    out_arrs = r["fn"](*concat_in, *concat_zeros)
    return [
        {
            name: np.asarray(out_arrs[i]).reshape(
                N_CORES, *r["out_avals"][i].shape
            )[c]
            for i, name in enumerate(r["out_names"])
        }
        for c in range(N_CORES)
    ]


def kernel(data1, data2):
    in_maps = prepare_inputs(data1, data2)
    results = run_hw(in_maps)
    out = np.stack([_extract(r["bands"]) for r in results])
    return out.astype(np.float32)
